# revision 12
# baseline (speedup 1.0000x reference)
"""DecoderLSTM Trainium2 kernel — tensor-parallel over gate columns.

Topology: 8 NeuronCores as 2 quads x 4 members (TP4 x DP2).
  - Quad q handles batch rows [256q, 256q+256); all 4 members share them.
  - Member m owns gate columns {g*1024 + [256m,256m+256) : g in i,f,g,o},
    i.e. hidden slice hm = [256m, 256m+256) of both LSTM layers, and rows
    hm of fc1 (K-sharded fc1 -> AllReduce of z partials).
  - All weights are SBUF-resident as fp16 hi/lo pairs (~14 MB/core); the
    embedding is folded into layer-0 input weights on the host in float64
    (xe @ W_ih0 == y @ (emb_W @ W_ih0)), with the layer-0 bias folded in
    as a 65th input row against a constant-one activation row.

Matmuls run as 3-pass fp16 (hi*hi + lo*hi + hi*lo, fp32 PSUM accumulate):
measured 3.6e-7 max rel err per matmul (fp32-level) at 3 cycles/row vs
fp32's 4.  States c0/c1 stay fp32 and local; h0/h1 cross cores as fp16
hi/lo pairs via AllGather; z crosses as fp32 via AllReduce.

Per-step comm (DRAM bounce collectives): AG(h0T pair 256KB), AR(z 1MB),
AG(h1T pair 256KB), software-pipelined so next-step gate matmuls cover
collective latency.

Self-contained: shapes/sharding hardcoded; reads nothing from disk.
"""
from contextlib import ExitStack

import numpy as np

import concourse.bass as bass
import concourse.tile as tile
from concourse import bacc, mybir
from concourse import bass_utils

F32 = mybir.dt.float32
F16 = mybir.dt.float16
AF = mybir.ActivationFunctionType
ALU = mybir.AluOpType

B, D, H, T_FULL = 512, 64, 1024, 96
NC = 8
B2 = 256          # batch rows per quad
MC = 2            # 128-row chunks of B2
HS = 256          # hidden shard per member
G = 1024          # gate columns per member (4 * HS)
KT = H // 128     # 8 k-tiles over H
LN_EPS = 1e-5
GROUPS = [[0, 1, 2, 3], [4, 5, 6, 7]]

_cache = {}


def _emit(ctx: ExitStack, tc: tile.TileContext, io: dict, t_steps: int,
          trivial_ln: bool):
    nc = tc.nc

    res = ctx.enter_context(tc.tile_pool(name="res", bufs=1))
    state = ctx.enter_context(tc.tile_pool(name="state", bufs=1))
    work = ctx.enter_context(tc.tile_pool(name="work", bufs=1))
    psum = ctx.enter_context(tc.tile_pool(name="psum", bufs=4, space="PSUM"))
    tpsum = ctx.enter_context(tc.tile_pool(name="tpsum", bufs=3, space="PSUM"))
    dram = ctx.enter_context(tc.tile_pool(name="dram", bufs=2, space="DRAM"))

    # ---- resident weights (fp16 hi/lo pairs) and constants ----
    wemb = [res.tile([65, G], F16, name=f"wemb{i}") for i in range(2)]
    whh0 = [res.tile([128, KT, G], F16, name=f"whh0{i}") for i in range(2)]
    wih1 = [res.tile([128, KT, G], F16, name=f"wih1{i}") for i in range(2)]
    whh1 = [res.tile([128, KT, G], F16, name=f"whh1{i}") for i in range(2)]
    wfc1 = [res.tile([128, 2, H], F16, name=f"wfc1{i}") for i in range(2)]
    wfc2 = [res.tile([128, KT, 64], F16, name=f"wfc2{i}") for i in range(2)]
    b1g = res.tile([128, G], F32)
    fc1b = res.tile([128, H], F32)
    if not trivial_ln:
        lng = res.tile([128, H], F32)
        lnb = res.tile([128, H], F32)
    fc2b = res.tile([128, 64], F32)
    ident = res.tile([128, 128], F32)
    for i in range(2):
        sfx = ["hi", "lo"][i]
        nc.sync.dma_start(wemb[i][:], io[f"wemb_{sfx}"].ap())
        for t_, n_ in [(whh0, "whh0"), (wih1, "wih1"), (whh1, "whh1")]:
            nc.sync.dma_start(
                t_[i][:],
                io[f"{n_}_{sfx}"].ap().rearrange("(k p) n -> p k n", p=128))
        nc.sync.dma_start(
            wfc1[i][:],
            io[f"wfc1_{sfx}"].ap().rearrange("(k p) n -> p k n", p=128))
        nc.sync.dma_start(
            wfc2[i][:],
            io[f"wfc2_{sfx}"].ap().rearrange("(k p) n -> p k n", p=128))
    consts = [(b1g, "b1g"), (fc1b, "fc1b"), (fc2b, "fc2b"), (ident, "ident")]
    if not trivial_ln:
        consts += [(lng, "lng"), (lnb, "lnb")]
    for t_, n_ in consts:
        nc.sync.dma_start(t_[:], io[n_].ap())
    eps_t = res.tile([128, 1], F32)
    nc.vector.memset(eps_t[:], LN_EPS)

    # ---- initial state ----
    yT = [state.tile([65, B2], F16, tag=f"yT{i}", name=f"yT_init{i}")
          for i in range(2)]
    h0T = [state.tile([128, KT, B2], F16, tag=f"h0T{i}", name=f"h0T_init{i}")
           for i in range(2)]
    h1T = [state.tile([128, KT, B2], F16, tag=f"h1T{i}", name=f"h1T_init{i}")
           for i in range(2)]
    c0 = state.tile([128, MC, HS], F32, tag="c0", bufs=2)
    c1 = state.tile([128, MC, HS], F32, tag="c1", bufs=2)
    for i in range(2):
        sfx = ["hi", "lo"][i]
        nc.sync.dma_start(yT[i][:], io[f"x0T_{sfx}"].ap())
        nc.sync.dma_start(
            h0T[i][:],
            io[f"h0T0_{sfx}"].ap().rearrange("(k p) b -> p k b", p=128))
        nc.sync.dma_start(
            h1T[i][:],
            io[f"h1T0_{sfx}"].ap().rearrange("(k p) b -> p k b", p=128))
    nc.sync.dma_start(
        c0[:], io["c00"].ap().rearrange("(mc p) h -> p mc h", p=128))
    nc.sync.dma_start(
        c1[:], io["c10"].ap().rearrange("(mc p) h -> p mc h", p=128))

    def mm3(p, lhsT, rhs, start, stop, n0, n1):
        """3-pass fp16 matmul accumulate: hi*hi + lo*hi + hi*lo."""
        nc.tensor.matmul(p, lhsT[0], rhs[0][:, n0:n1], start=start,
                         stop=False, skip_group_check=True)
        nc.tensor.matmul(p, lhsT[1], rhs[0][:, n0:n1], start=False,
                         stop=False, skip_group_check=True)
        nc.tensor.matmul(p, lhsT[0], rhs[1][:, n0:n1], start=False,
                         stop=stop, skip_group_check=True)

    def gates_hh(hT, w, tag):
        """Open a 4-bank psum group with the recurrent contribution."""
        ps = [[psum.tile([128, 512], F32, tag="p512",
                         name=f"{tag}_{mc}_{nk}") for nk in range(2)]
              for mc in range(2)]
        for mc in range(2):
            for nk in range(2):
                for k in range(KT):
                    mm3(ps[mc][nk][:],
                        [hT[i][:, k, mc * 128:(mc + 1) * 128] for i in range(2)],
                        [w[i][:, k, :] for i in range(2)],
                        start=(k == 0), stop=False,
                        n0=nk * 512, n1=(nk + 1) * 512)
        return ps

    def gates0_ih(ps, yT_pair):
        """Close the gates0 group with the folded-embedding input part
        (65 rows: 64 of y plus a ones-row carrying the bias)."""
        for mc in range(2):
            for nk in range(2):
                mm3(ps[mc][nk][:],
                    [yT_pair[i][:, mc * 128:(mc + 1) * 128] for i in range(2)],
                    wemb, start=False, stop=True,
                    n0=nk * 512, n1=(nk + 1) * 512)

    def gates1_ih(ps, h0T_new):
        for mc in range(2):
            for nk in range(2):
                for k in range(KT):
                    mm3(ps[mc][nk][:],
                        [h0T_new[i][:, k, mc * 128:(mc + 1) * 128]
                         for i in range(2)],
                        [wih1[i][:, k, :] for i in range(2)],
                        start=False, stop=(k == KT - 1),
                        n0=nk * 512, n1=(nk + 1) * 512)

    def evac_nonlin_cell(ps, bias, c_prev, layer):
        """psum -> gts (+bias), i|f sigmoid, g tanh, o sigmoid, cell update.
        Returns (h_sh [128,MC,HS] f32, c_new, g_t)."""
        g_t = work.tile([128, MC, G], F32, tag="gts", name=f"gts_l{layer}",
                        bufs=2)
        c_new = state.tile([128, MC, HS], F32, tag=f"c{layer}",
                           name=f"c{layer}n", bufs=2)
        h_sh = work.tile([128, MC, HS], F32, tag=f"h{layer}",
                         name=f"h{layer}sh", bufs=1)
        for mc in range(2):
            for nk in range(2):
                sl = slice(nk * 512, (nk + 1) * 512)
                if bias is None:
                    nc.vector.tensor_copy(g_t[:, mc, sl], ps[mc][nk][:])
                else:
                    nc.vector.tensor_add(g_t[:, mc, sl], ps[mc][nk][:],
                                         bias[:, sl])
            nc.scalar.activation(g_t[:, mc, 0:512], g_t[:, mc, 0:512],
                                 AF.Sigmoid)
            nc.scalar.activation(g_t[:, mc, 512:768], g_t[:, mc, 512:768],
                                 AF.Tanh)
            nc.scalar.activation(g_t[:, mc, 768:1024], g_t[:, mc, 768:1024],
                                 AF.Sigmoid)
            tmp = work.tile([128, HS], F32, tag="ctmp", name=f"ct{layer}{mc}",
                            bufs=2)
            tanhc = work.tile([128, HS], F32, tag="tanhc",
                              name=f"th{layer}{mc}", bufs=2)
            nc.vector.tensor_mul(tmp[:], g_t[:, mc, 256:512], c_prev[:, mc, :])
            nc.vector.tensor_mul(c_new[:, mc, :], g_t[:, mc, 0:256],
                                 g_t[:, mc, 512:768])
            nc.vector.tensor_add(c_new[:, mc, :], c_new[:, mc, :], tmp[:])
            nc.scalar.activation(tanhc[:], c_new[:, mc, :], AF.Tanh)
            nc.vector.tensor_mul(h_sh[:, mc, :], g_t[:, mc, 768:1024],
                                 tanhc[:])
        return h_sh, c_new, g_t

    def transpose_split(h_sh, tag):
        """[128,MC,HS] f32 -> transposed fp16 pair [128, 2(hb), B2]."""
        pair = [work.tile([128, 2, B2], F16, tag=f"{tag}{i}",
                          name=f"{tag}p{i}", bufs=1) for i in range(2)]
        for mc in range(2):
            for hb in range(2):
                tp = tpsum.tile([128, 128], F32, tag="tp", name=f"tp_{tag}")
                nc.tensor.transpose(tp[:], h_sh[:, mc, hb * 128:(hb + 1) * 128],
                                    ident[:])
                rt = work.tile([128, 128], F32, tag="rt", name=f"rt_{tag}",
                               bufs=2)
                bs = slice(mc * 128, (mc + 1) * 128)
                nc.scalar.activation(pair[0][:, hb, bs], tp[:], AF.Copy)
                nc.vector.tensor_sub(rt[:], tp[:], pair[0][:, hb, bs])
                nc.gpsimd.tensor_copy(pair[1][:, hb, bs], rt[:])
        return pair

    def ag_pair(pair, layer, t):
        """AllGather the transposed shard pair -> full [128, KT, B2] pair."""
        gin = dram.tile([2, 128, 2, B2], F16, tag=f"gin{layer}",
                        name=f"gin{layer}_{t}")
        gout = dram.tile([4, 2, 128, 2, B2], F16, tag=f"gout{layer}",
                         name=f"gout{layer}_{t}")
        for i in range(2):
            nc.sync.dma_start(gin[i], pair[i][:])
        nc.gpsimd.collective_compute(
            "AllGather", ALU.bypass, replica_groups=GROUPS,
            ins=[gin.opt()], outs=[gout.opt()])
        full = [state.tile([128, KT, B2], F16, tag=f"h{layer}T{i}",
                           name=f"h{layer}Tn{i}") for i in range(2)]
        for i in range(2):
            for m_i in range(4):
                nc.sync.dma_start(full[i][:, 2 * m_i:2 * m_i + 2, :],
                                  gout[m_i, i])
        return full

    for t in range(t_steps):
        # ---- finish gates0(t): psum group was opened earlier ----
        if t == 0:
            g0 = gates_hh(h0T, whh0, "g0")
            gates0_ih(g0, yT)
        h0_sh, c0, _ = evac_nonlin_cell(g0, None, c0, 0)
        h0T_sh = transpose_split(h0_sh, "h0s")
        h0T = ag_pair(h0T_sh, 0, t)

        # ---- gates1(t): hh (ready) then ih (waits AG0) ----
        g1 = gates_hh(h1T, whh1, "g1")
        gates1_ih(g1, h0T)
        h1_sh, c1, gt1 = evac_nonlin_cell(g1, b1g, c1, 1)
        h1T_sh = transpose_split(h1_sh, "h1s")

        # ---- fc1 K-shard partial: z_part = h1T_sh.T @ wfc1 ----
        zps = [[psum.tile([128, 512], F32, tag="p512", name=f"z_{mc}_{nk}")
                for nk in range(2)] for mc in range(2)]
        for mc in range(2):
            for nk in range(2):
                for k in range(2):
                    mm3(zps[mc][nk][:],
                        [h1T_sh[i][:, k, mc * 128:(mc + 1) * 128]
                         for i in range(2)],
                        [wfc1[i][:, k, :] for i in range(2)],
                        start=(k == 0), stop=(k == 1),
                        n0=nk * 512, n1=(nk + 1) * 512)
        zp = work.tile([128, MC, H], F32, tag="gts", name="zp", bufs=2)
        for mc in range(2):
            for nk in range(2):
                nc.vector.tensor_copy(zp[:, mc, nk * 512:(nk + 1) * 512],
                                      zps[mc][nk][:])

        # ---- merged AllGather: z partial (f32 bytes) + h1T pair ----
        gm = dram.tile([2, 128, 2048 + 512], F16, tag="gm", name=f"gm{t}")
        gmo = dram.tile([4, 2, 128, 2048 + 512], F16, tag="gmo",
                        name=f"gmo{t}")
        nc.sync.dma_start(
            gm[:, :, 0:2048].rearrange("mc p n -> p mc n"),
            zp[:].bitcast(F16))
        for i in range(2):
            nc.sync.dma_start(gm[i, :, 2048:2560].rearrange("p (hb b) -> p hb b", hb=2),
                              h1T_sh[i][:])
        nc.gpsimd.collective_compute(
            "AllGather", ALU.bypass, replica_groups=GROUPS,
            ins=[gm.opt()], outs=[gmo.opt()])
        h1T = [state.tile([128, KT, B2], F16, tag=f"h1T{i}",
                          name=f"h1Tn{i}_{t}") for i in range(2)]
        for i in range(2):
            for m_i in range(4):
                nc.sync.dma_start(
                    h1T[i][:, 2 * m_i:2 * m_i + 2, :],
                    gmo[m_i, i, :, 2048:2560].rearrange("p (hb b) -> p hb b",
                                                        hb=2))

        # ---- open gates0(t+1) hh while the gather is in flight ----
        if t + 1 < t_steps:
            g0 = gates_hh(h0T, whh0, "g0")

        # ---- z = sum of 4 gathered partials, + fc1 bias ----
        z = work.tile([128, MC, H], F32, tag="z", name=f"z{t}", bufs=1)
        nc.sync.dma_start(
            z[:], gmo[0, :, :, 0:2048].rearrange("mc p n -> p mc n")
            .bitcast(F32))
        reluT = [work.tile([128, KT, B2], F16, tag=f"reluT{i}",
                           name=f"rT{i}_{t}", bufs=1) for i in range(2)]
        for m_i in range(1, 4):
            for mc in range(2):
                zb = work.tile([128, 2048], F16, tag="zb",
                               name=f"zb{t}{m_i}{mc}", bufs=2)
                nc.sync.dma_start(zb[:], gmo[m_i, mc, :, 0:2048])
                nc.vector.tensor_add(z[:, mc, :], z[:, mc, :],
                                     zb[:].bitcast(F32))
        for mc in range(2):
            s = work.tile([128, 1], F32, tag="s1", bufs=8, name=f"s{t}{mc}")
            mu = work.tile([128, 1], F32, tag="s1", bufs=8, name=f"mu{t}{mc}")
            negmu = work.tile([128, 1], F32, tag="s1", bufs=8,
                              name=f"nmu{t}{mc}")
            sqs = work.tile([128, 1], F32, tag="s1", bufs=8, name=f"sqs{t}{mc}")
            va = work.tile([128, 1], F32, tag="s1", bufs=8, name=f"va{t}{mc}")
            sv = work.tile([128, 1], F32, tag="s1", bufs=8, name=f"sv{t}{mc}")
            rstd = work.tile([128, 1], F32, tag="s1", bufs=8,
                             name=f"rs{t}{mc}")
            nc.vector.tensor_add(z[:, mc, :], z[:, mc, :], fc1b[:])
            nc.vector.reduce_sum(s[:], z[:, mc, :], axis=mybir.AxisListType.X)
            nc.vector.tensor_scalar_mul(negmu[:], s[:], -1.0 / H)
            nc.vector.tensor_scalar_mul(mu[:], s[:], 1.0 / H)
            # Square scratch: reuse the dead layer-1 gate tile
            nc.scalar.activation(gt1[:, mc, :], z[:, mc, :], AF.Square,
                                 bias=negmu[:], accum_out=sqs[:])
            nc.vector.tensor_scalar_mul(va[:], sqs[:], 1.0 / H)
            nc.scalar.activation(sv[:], va[:], AF.Sqrt, bias=eps_t[:])
            nc.vector.reciprocal(rstd[:], sv[:])
            nc.vector.tensor_scalar(z[:, mc, :], z[:, mc, :], mu[:], rstd[:],
                                    ALU.subtract, ALU.mult)
            if not trivial_ln:
                nc.vector.tensor_mul(z[:, mc, :], z[:, mc, :], lng[:])
                nc.vector.tensor_add(z[:, mc, :], z[:, mc, :], lnb[:])
            # transpose (pre-relu) -> relu fused into the psum evacuation
            for hb in range(KT):
                tp = tpsum.tile([128, 128], F32, tag="tp", name=f"tp_r{t}")
                nc.tensor.transpose(
                    tp[:], z[:, mc, hb * 128:(hb + 1) * 128], ident[:])
                rt = work.tile([128, 128], F32, tag="rt", name=f"rt_r{t}",
                               bufs=2)
                bs = slice(mc * 128, (mc + 1) * 128)
                nc.scalar.activation(reluT[0][:, hb, bs], tp[:], AF.Relu)
                nc.vector.scalar_tensor_tensor(
                    rt[:], tp[:], 0.0, reluT[0][:, hb, bs],
                    ALU.max, ALU.subtract)
                nc.gpsimd.tensor_copy(reluT[1][:, hb, bs], rt[:])

        # ---- fc2 -> y [128, MC, 64], output DMA ----
        y = work.tile([128, MC, 64], F32, tag="y", name=f"y{t}", bufs=1)
        for mc in range(2):
            yp = tpsum.tile([128, 128], F32, tag="tp", name=f"yp{t}")
            for k in range(KT):
                mm3(yp[:, 0:64],
                    [reluT[i][:, k, mc * 128:(mc + 1) * 128] for i in range(2)],
                    [wfc2[i][:, k, :] for i in range(2)],
                    start=(k == 0), stop=(k == KT - 1), n0=0, n1=64)
            nc.vector.tensor_add(y[:, mc, :], yp[:, 0:64], fc2b[:, 0:64])
        nc.sync.dma_start(
            io["ys"].ap()[t].rearrange("(mc p) d -> p mc d", p=128), y[:])

        # ---- yT pair for next step's gates0 input part ----
        if t + 1 < t_steps:
            yTn = [state.tile([65, B2], F16, tag=f"yT{i}", name=f"yTn{i}_{t}")
                   for i in range(2)]
            yTf = work.tile([64, B2], F32, tag="yTf", name=f"yTf{t}", bufs=1)
            for mc in range(2):
                ytp = tpsum.tile([128, 128], F32, tag="tp", name=f"ytp{t}")
                nc.tensor.transpose(ytp[0:64, :], y[:, mc, :], ident[:])
                nc.vector.tensor_copy(yTf[:, mc * 128:(mc + 1) * 128],
                                      ytp[0:64, 0:128])
            nc.scalar.activation(yTn[0][0:64, :], yTf[:], AF.Copy)
            rty = work.tile([64, B2], F32, tag="rty", name=f"rty{t}", bufs=1)
            nc.vector.tensor_sub(rty[:], yTf[:], yTn[0][0:64, :])
            nc.gpsimd.tensor_copy(yTn[1][0:64, :], rty[:])
            nc.vector.memset(yTn[0][64:65, :], 1.0)
            nc.vector.memset(yTn[1][64:65, :], 0.0)
            yT = yTn
            gates0_ih(g0, yT)


def build(t_steps=T_FULL, trivial_ln=True):
    key = (t_steps, trivial_ln)
    if key in _cache:
        return _cache[key]
    nc = bacc.Bacc("TRN2", target_bir_lowering=False, debug=False,
                   num_devices=NC)
    io = {}
    inputs = [
        ("wemb_hi", (65, G), F16), ("wemb_lo", (65, G), F16),
        ("whh0_hi", (H, G), F16), ("whh0_lo", (H, G), F16),
        ("wih1_hi", (H, G), F16), ("wih1_lo", (H, G), F16),
        ("whh1_hi", (H, G), F16), ("whh1_lo", (H, G), F16),
        ("wfc1_hi", (HS, H), F16), ("wfc1_lo", (HS, H), F16),
        ("wfc2_hi", (H, 64), F16), ("wfc2_lo", (H, 64), F16),
        ("b1g", (128, G), F32),
        ("fc1b", (128, H), F32), ("fc2b", (128, 64), F32),
        ("ident", (128, 128), F32),
        ("x0T_hi", (65, B2), F16), ("x0T_lo", (65, B2), F16),
        ("h0T0_hi", (H, B2), F16), ("h0T0_lo", (H, B2), F16),
        ("h1T0_hi", (H, B2), F16), ("h1T0_lo", (H, B2), F16),
        ("c00", (B2, HS), F32), ("c10", (B2, HS), F32),
    ]
    if not trivial_ln:
        inputs += [("lng", (128, H), F32), ("lnb", (128, H), F32)]
    for name, shape, dt in inputs:
        io[name] = nc.dram_tensor(name, shape, dt, kind="ExternalInput")
    io["ys"] = nc.dram_tensor("ys", (t_steps, B2, D), F32,
                              kind="ExternalOutput")
    with tile.TileContext(nc) as tc:
        with ExitStack() as ctx:
            _emit(ctx, tc, io, t_steps, trivial_ln)
    nc.compile()
    _cache[key] = (nc, io)
    return nc, io


def _split16(x):
    hi = x.astype(np.float16)
    lo = (x - hi.astype(np.float64)).astype(np.float16)
    return np.ascontiguousarray(hi), np.ascontiguousarray(lo)


def _trivial_ln(inputs):
    return bool(np.all(np.asarray(inputs["ln_g"]) == 1.0)
                and np.all(np.asarray(inputs["ln_b"]) == 0.0))


def make_in_maps(inputs, trivial_ln=True):
    f64 = lambda k: np.asarray(inputs[k]).astype(np.float64)
    emb_W, emb_b = f64("emb_W"), f64("emb_b")
    W_ih0 = f64("W_ih0")
    Wemb = emb_W @ W_ih0                     # [64, 4096]
    b0 = f64("b_ih0") + f64("b_hh0") + emb_b @ W_ih0
    b1 = f64("b_ih1") + f64("b_hh1")
    W_hh0, W_ih1, W_hh1 = f64("W_hh0"), f64("W_ih1"), f64("W_hh1")
    fc1_W, fc2_W = f64("fc1_W"), f64("fc2_W")
    x0 = f64("x_0")
    hn, cn = f64("h_n"), f64("c_n")
    ones = np.ones((1, B2), np.float64)

    bc = lambda v: np.tile(v.astype(np.float32)[None, :], (128, 1))
    in_maps = []
    for c in range(NC):
        q, m = c // 4, c % 4
        bs = slice(q * B2, (q + 1) * B2)
        cols = np.concatenate(
            [np.arange(g * H + m * HS, g * H + (m + 1) * HS) for g in range(4)])
        m_ = {}
        wemb_aug = np.vstack([Wemb[:, cols], b0[cols][None, :]])  # [65, G]
        for name, w in [("wemb", wemb_aug), ("whh0", W_hh0[:, cols]),
                        ("wih1", W_ih1[:, cols]), ("whh1", W_hh1[:, cols]),
                        ("wfc1", fc1_W[m * HS:(m + 1) * HS, :]),
                        ("wfc2", fc2_W)]:
            m_[f"{name}_hi"], m_[f"{name}_lo"] = _split16(w)
        m_["b1g"] = bc(b1[cols])
        m_["fc1b"] = bc(f64("fc1_b"))
        if not trivial_ln:
            m_["lng"] = bc(f64("ln_g"))
            m_["lnb"] = bc(f64("ln_b"))
        m_["fc2b"] = bc(f64("fc2_b"))
        m_["ident"] = np.eye(128, dtype=np.float32)
        m_["x0T_hi"], m_["x0T_lo"] = _split16(
            np.vstack([x0[bs].T, ones]))
        m_["h0T0_hi"], m_["h0T0_lo"] = _split16(hn[0][bs].T)
        m_["h1T0_hi"], m_["h1T0_lo"] = _split16(hn[1][bs].T)
        m_["c00"] = np.ascontiguousarray(
            cn[0][bs, m * HS:(m + 1) * HS].astype(np.float32))
        m_["c10"] = np.ascontiguousarray(
            cn[1][bs, m * HS:(m + 1) * HS].astype(np.float32))
        in_maps.append(m_)
    return in_maps


def kernel(**inputs):
    t_steps = int(inputs.get("forecast_window", T_FULL))
    triv = _trivial_ln(inputs)
    nc, io = build(t_steps, triv)
    in_maps = make_in_maps(inputs, triv)
    r = bass_utils.run_bass_kernel_spmd(nc, in_maps, core_ids=list(range(NC)))
    out = np.empty((B, t_steps, D), np.float32)
    for q in range(2):
        ys = r.results[q * 4]["ys"]            # [t, B2, D]
        out[q * B2:(q + 1) * B2] = ys.transpose(1, 0, 2)
    return out


# revision 14
# speedup vs baseline: 1.2777x; 1.2777x over previous
"""DecoderLSTM Trainium2 kernel — tensor-parallel over gate columns.

Topology: 8 NeuronCores as 2 quads x 4 members (TP4 x DP2).
  - Quad q handles batch rows [256q, 256q+256); all 4 members share them.
  - Member m owns gate columns {g*1024 + [256m,256m+256) : g in i,f,g,o},
    i.e. hidden slice hm = [256m, 256m+256) of both LSTM layers, and rows
    hm of fc1 (K-sharded fc1 -> AllReduce of z partials).
  - All weights are SBUF-resident as fp16 hi/lo pairs (~14 MB/core); the
    embedding is folded into layer-0 input weights on the host in float64
    (xe @ W_ih0 == y @ (emb_W @ W_ih0)), with the layer-0 bias folded in
    as a 65th input row against a constant-one activation row.

Matmuls run as 3-pass fp16 (hi*hi + lo*hi + hi*lo, fp32 PSUM accumulate):
measured 3.6e-7 max rel err per matmul (fp32-level) at 3 cycles/row vs
fp32's 4.  States c0/c1 stay fp32 and local; h0/h1 cross cores as fp16
hi/lo pairs via AllGather; z crosses as fp32 via AllReduce.

Per-step comm (DRAM bounce collectives): AG(h0T pair 256KB), AR(z 1MB),
AG(h1T pair 256KB), software-pipelined so next-step gate matmuls cover
collective latency.

Self-contained: shapes/sharding hardcoded; reads nothing from disk.
"""
from contextlib import ExitStack

import numpy as np

import concourse.bass as bass
import concourse.tile as tile
from concourse import bacc, mybir
from concourse import bass_utils

F32 = mybir.dt.float32
F16 = mybir.dt.float16
AF = mybir.ActivationFunctionType
ALU = mybir.AluOpType

B, D, H, T_FULL = 512, 64, 1024, 96
NC = 8
B2 = 256          # batch rows per quad
MC = 2            # 128-row chunks of B2
HS = 256          # hidden shard per member
G = 1024          # gate columns per member (4 * HS)
KT = H // 128     # 8 k-tiles over H
LN_EPS = 1e-5
GROUPS = [[0, 1, 2, 3], [4, 5, 6, 7]]

_cache = {}


def _emit(ctx: ExitStack, tc: tile.TileContext, io: dict, t_steps: int,
          trivial_ln: bool):
    nc = tc.nc

    res = ctx.enter_context(tc.tile_pool(name="res", bufs=1))
    state = ctx.enter_context(tc.tile_pool(name="state", bufs=1))
    work = ctx.enter_context(tc.tile_pool(name="work", bufs=1))
    psum = ctx.enter_context(tc.tile_pool(name="psum", bufs=4, space="PSUM"))
    tpsum = ctx.enter_context(tc.tile_pool(name="tpsum", bufs=3, space="PSUM"))
    dram = ctx.enter_context(tc.tile_pool(name="dram", bufs=2, space="DRAM"))

    # ---- resident weights (fp16 hi/lo pairs) and constants ----
    wemb = [res.tile([65, G], F16, name=f"wemb{i}") for i in range(2)]
    whh0 = [res.tile([128, KT, G], F16, name=f"whh0{i}") for i in range(2)]
    wih1 = [res.tile([128, KT, G], F16, name=f"wih1{i}") for i in range(2)]
    whh1 = [res.tile([128, KT, G], F16, name=f"whh1{i}") for i in range(2)]
    wfc1 = [res.tile([128, 2, H], F16, name=f"wfc1{i}") for i in range(2)]
    wfc2 = [res.tile([128, KT, 64], F16, name=f"wfc2{i}") for i in range(2)]
    b1g = res.tile([128, G], F32)
    fc1b = res.tile([128, H], F32)
    if not trivial_ln:
        lng = res.tile([128, H], F32)
        lnb = res.tile([128, H], F32)
    fc2b = res.tile([128, 64], F32)
    ident = res.tile([128, 128], F32)
    for i in range(2):
        sfx = ["hi", "lo"][i]
        nc.sync.dma_start(wemb[i][:], io[f"wemb_{sfx}"].ap())
        for t_, n_ in [(whh0, "whh0"), (wih1, "wih1"), (whh1, "whh1")]:
            nc.sync.dma_start(
                t_[i][:],
                io[f"{n_}_{sfx}"].ap().rearrange("(k p) n -> p k n", p=128))
        nc.sync.dma_start(
            wfc1[i][:],
            io[f"wfc1_{sfx}"].ap().rearrange("(k p) n -> p k n", p=128))
        nc.sync.dma_start(
            wfc2[i][:],
            io[f"wfc2_{sfx}"].ap().rearrange("(k p) n -> p k n", p=128))
    consts = [(b1g, "b1g"), (fc1b, "fc1b"), (fc2b, "fc2b"), (ident, "ident")]
    if not trivial_ln:
        consts += [(lng, "lng"), (lnb, "lnb")]
    for t_, n_ in consts:
        nc.sync.dma_start(t_[:], io[n_].ap())
    eps_t = res.tile([128, 1], F32)
    nc.vector.memset(eps_t[:], LN_EPS)

    # ---- initial state ----
    yT = [state.tile([65, B2], F16, tag=f"yT{i}", name=f"yT_init{i}")
          for i in range(2)]
    h0T = [state.tile([128, KT, B2], F16, tag=f"h0T{i}", name=f"h0T_init{i}")
           for i in range(2)]
    h1T = [state.tile([128, KT, B2], F16, tag=f"h1T{i}", name=f"h1T_init{i}")
           for i in range(2)]
    c0 = state.tile([128, MC, HS], F32, tag="c0", bufs=2)
    c1 = state.tile([128, MC, HS], F32, tag="c1", bufs=2)
    for i in range(2):
        sfx = ["hi", "lo"][i]
        nc.sync.dma_start(yT[i][:], io[f"x0T_{sfx}"].ap())
        nc.sync.dma_start(
            h0T[i][:],
            io[f"h0T0_{sfx}"].ap().rearrange("(k p) b -> p k b", p=128))
        nc.sync.dma_start(
            h1T[i][:],
            io[f"h1T0_{sfx}"].ap().rearrange("(k p) b -> p k b", p=128))
    nc.sync.dma_start(
        c0[:], io["c00"].ap().rearrange("(mc p) h -> p mc h", p=128))
    nc.sync.dma_start(
        c1[:], io["c10"].ap().rearrange("(mc p) h -> p mc h", p=128))

    def mm3(p, lhsT, rhs, start, stop, n0, n1):
        """3-pass fp16 matmul accumulate: hi*hi + lo*hi + hi*lo."""
        nc.tensor.matmul(p, lhsT[0], rhs[0][:, n0:n1], start=start,
                         stop=False, skip_group_check=True)
        nc.tensor.matmul(p, lhsT[1], rhs[0][:, n0:n1], start=False,
                         stop=False, skip_group_check=True)
        nc.tensor.matmul(p, lhsT[0], rhs[1][:, n0:n1], start=False,
                         stop=stop, skip_group_check=True)

    def gates_hh(hT, w, tag):
        """Open a 4-bank psum group with the recurrent contribution."""
        ps = [[psum.tile([128, 512], F32, tag="p512",
                         name=f"{tag}_{mc}_{nk}") for nk in range(2)]
              for mc in range(2)]
        for mc in range(2):
            for nk in range(2):
                for k in range(KT):
                    mm3(ps[mc][nk][:],
                        [hT[i][:, k, mc * 128:(mc + 1) * 128] for i in range(2)],
                        [w[i][:, k, :] for i in range(2)],
                        start=(k == 0), stop=False,
                        n0=nk * 512, n1=(nk + 1) * 512)
        return ps

    def gates0_ih(ps, yT_pair):
        """Close the gates0 group with the folded-embedding input part
        (65 rows: 64 of y plus a ones-row carrying the bias)."""
        for mc in range(2):
            for nk in range(2):
                mm3(ps[mc][nk][:],
                    [yT_pair[i][:, mc * 128:(mc + 1) * 128] for i in range(2)],
                    wemb, start=False, stop=True,
                    n0=nk * 512, n1=(nk + 1) * 512)

    def gates1_ih(ps, h0T_new):
        for mc in range(2):
            for nk in range(2):
                for k in range(KT):
                    mm3(ps[mc][nk][:],
                        [h0T_new[i][:, k, mc * 128:(mc + 1) * 128]
                         for i in range(2)],
                        [wih1[i][:, k, :] for i in range(2)],
                        start=False, stop=(k == KT - 1),
                        n0=nk * 512, n1=(nk + 1) * 512)

    def evac_nonlin_cell(ps, bias, c_prev, layer):
        """psum -> gts (+bias), i|f sigmoid, g tanh, o sigmoid, cell update.
        Returns (h_sh [128,MC,HS] f32, c_new, g_t)."""
        g_t = work.tile([128, MC, G], F32, tag="gts", name=f"gts_l{layer}",
                        bufs=2)
        c_new = state.tile([128, MC, HS], F32, tag=f"c{layer}",
                           name=f"c{layer}n", bufs=2)
        h_sh = work.tile([128, MC, HS], F32, tag=f"h{layer}",
                         name=f"h{layer}sh", bufs=1)
        for mc in range(2):
            for nk in range(2):
                sl = slice(nk * 512, (nk + 1) * 512)
                if bias is None:
                    nc.vector.tensor_copy(g_t[:, mc, sl], ps[mc][nk][:])
                else:
                    nc.vector.tensor_add(g_t[:, mc, sl], ps[mc][nk][:],
                                         bias[:, sl])
            nc.scalar.activation(g_t[:, mc, 0:512], g_t[:, mc, 0:512],
                                 AF.Sigmoid)
            nc.scalar.activation(g_t[:, mc, 512:768], g_t[:, mc, 512:768],
                                 AF.Tanh)
            nc.scalar.activation(g_t[:, mc, 768:1024], g_t[:, mc, 768:1024],
                                 AF.Sigmoid)
            tmp = work.tile([128, HS], F32, tag="ctmp", name=f"ct{layer}{mc}",
                            bufs=2)
            tanhc = work.tile([128, HS], F32, tag="tanhc",
                              name=f"th{layer}{mc}", bufs=2)
            nc.vector.tensor_mul(tmp[:], g_t[:, mc, 256:512], c_prev[:, mc, :])
            nc.vector.tensor_mul(c_new[:, mc, :], g_t[:, mc, 0:256],
                                 g_t[:, mc, 512:768])
            nc.vector.tensor_add(c_new[:, mc, :], c_new[:, mc, :], tmp[:])
            nc.scalar.activation(tanhc[:], c_new[:, mc, :], AF.Tanh)
            nc.vector.tensor_mul(h_sh[:, mc, :], g_t[:, mc, 768:1024],
                                 tanhc[:])
        return h_sh, c_new, g_t

    def transpose_split(h_sh, tag):
        """[128,MC,HS] f32 -> transposed fp16 pair [128, 2(hb), B2]."""
        pair = [work.tile([128, 2, B2], F16, tag=f"{tag}{i}",
                          name=f"{tag}p{i}", bufs=1) for i in range(2)]
        for mc in range(2):
            for hb in range(2):
                tp = tpsum.tile([128, 128], F32, tag="tp", name=f"tp_{tag}")
                nc.tensor.transpose(tp[:], h_sh[:, mc, hb * 128:(hb + 1) * 128],
                                    ident[:])
                rt = work.tile([128, 128], F32, tag="rt", name=f"rt_{tag}",
                               bufs=2)
                bs = slice(mc * 128, (mc + 1) * 128)
                nc.scalar.activation(pair[0][:, hb, bs], tp[:], AF.Copy)
                nc.vector.tensor_sub(rt[:], tp[:], pair[0][:, hb, bs])
                nc.gpsimd.tensor_copy(pair[1][:, hb, bs], rt[:])
        return pair

    def ag_pair(pair, layer, t):
        """AllGather the transposed shard pair -> full [128, KT, B2] pair."""
        gin = dram.tile([2, 128, 2, B2], F16, tag=f"gin{layer}",
                        name=f"gin{layer}_{t}")
        gout = dram.tile([4, 2, 128, 2, B2], F16, tag=f"gout{layer}",
                         name=f"gout{layer}_{t}")
        for i in range(2):
            nc.sync.dma_start(gin[i], pair[i][:])
        nc.gpsimd.collective_compute(
            "AllGather", ALU.bypass, replica_groups=GROUPS,
            ins=[gin.opt()], outs=[gout.opt()])
        full = [state.tile([128, KT, B2], F16, tag=f"h{layer}T{i}",
                           name=f"h{layer}Tn{i}") for i in range(2)]
        for i in range(2):
            for m_i in range(4):
                nc.sync.dma_start(full[i][:, 2 * m_i:2 * m_i + 2, :],
                                  gout[m_i, i])
        return full

    for t in range(t_steps):
        # ---- finish gates0(t): psum group was opened earlier ----
        if t == 0:
            g0 = gates_hh(h0T, whh0, "g0")
            gates0_ih(g0, yT)
        h0_sh, c0, _ = evac_nonlin_cell(g0, None, c0, 0)
        h0T_sh = transpose_split(h0_sh, "h0s")
        h0T = ag_pair(h0T_sh, 0, t)

        # ---- gates1(t): hh (ready) then ih (waits AG0) ----
        g1 = gates_hh(h1T, whh1, "g1")
        gates1_ih(g1, h0T)
        h1_sh, c1, gt1 = evac_nonlin_cell(g1, b1g, c1, 1)
        h1T_sh = transpose_split(h1_sh, "h1s")

        # ---- fc1 K-shard partial: z_part = h1T_sh.T @ wfc1 ----
        zps = [[psum.tile([128, 512], F32, tag="p512", name=f"z_{mc}_{nk}")
                for nk in range(2)] for mc in range(2)]
        for mc in range(2):
            for nk in range(2):
                for k in range(2):
                    mm3(zps[mc][nk][:],
                        [h1T_sh[i][:, k, mc * 128:(mc + 1) * 128]
                         for i in range(2)],
                        [wfc1[i][:, k, :] for i in range(2)],
                        start=(k == 0), stop=(k == 1),
                        n0=nk * 512, n1=(nk + 1) * 512)
        zp = work.tile([128, MC, H], F32, tag="gts", name="zp", bufs=2)
        for mc in range(2):
            for nk in range(2):
                nc.vector.tensor_copy(zp[:, mc, nk * 512:(nk + 1) * 512],
                                      zps[mc][nk][:])

        # ---- AR(z) in-network, then AG(h1) ----
        rin = dram.tile([2, 128, H], F32, tag="rin", name=f"rin{t}")
        rout = dram.tile([2, 128, H], F32, tag="rout", name=f"rout{t}")
        nc.sync.dma_start(rin[:].rearrange("mc p n -> p mc n"), zp[:])
        nc.gpsimd.collective_compute(
            "AllReduce", ALU.add, replica_groups=GROUPS,
            ins=[rin.opt()], outs=[rout.opt()])
        h1T = ag_pair(h1T_sh, 1, t)

        # ---- open gates0(t+1) hh while AR(z) is in flight ----
        if t + 1 < t_steps:
            g0 = gates_hh(h0T, whh0, "g0")

        # ---- z reload, + fc1 bias, LayerNorm, fused relu+transpose ----
        z = work.tile([128, MC, H], F32, tag="z", name=f"z{t}", bufs=1)
        nc.sync.dma_start(z[:], rout[:].rearrange("mc p n -> p mc n"))
        reluT = [work.tile([128, KT, B2], F16, tag=f"reluT{i}",
                           name=f"rT{i}_{t}", bufs=1) for i in range(2)]
        for mc in range(2):
            s = work.tile([128, 1], F32, tag="s1", bufs=8, name=f"s{t}{mc}")
            mu = work.tile([128, 1], F32, tag="s1", bufs=8, name=f"mu{t}{mc}")
            negmu = work.tile([128, 1], F32, tag="s1", bufs=8,
                              name=f"nmu{t}{mc}")
            sqs = work.tile([128, 1], F32, tag="s1", bufs=8, name=f"sqs{t}{mc}")
            va = work.tile([128, 1], F32, tag="s1", bufs=8, name=f"va{t}{mc}")
            sv = work.tile([128, 1], F32, tag="s1", bufs=8, name=f"sv{t}{mc}")
            rstd = work.tile([128, 1], F32, tag="s1", bufs=8,
                             name=f"rs{t}{mc}")
            nc.vector.tensor_add(z[:, mc, :], z[:, mc, :], fc1b[:])
            nc.vector.reduce_sum(s[:], z[:, mc, :], axis=mybir.AxisListType.X)
            nc.vector.tensor_scalar_mul(negmu[:], s[:], -1.0 / H)
            nc.vector.tensor_scalar_mul(mu[:], s[:], 1.0 / H)
            # Square scratch: reuse the dead layer-1 gate tile
            nc.scalar.activation(gt1[:, mc, :], z[:, mc, :], AF.Square,
                                 bias=negmu[:], accum_out=sqs[:])
            nc.vector.tensor_scalar_mul(va[:], sqs[:], 1.0 / H)
            nc.scalar.activation(sv[:], va[:], AF.Sqrt, bias=eps_t[:])
            nc.vector.reciprocal(rstd[:], sv[:])
            nc.vector.tensor_scalar(z[:, mc, :], z[:, mc, :], mu[:], rstd[:],
                                    ALU.subtract, ALU.mult)
            if not trivial_ln:
                nc.vector.tensor_mul(z[:, mc, :], z[:, mc, :], lng[:])
                nc.vector.tensor_add(z[:, mc, :], z[:, mc, :], lnb[:])
            # transpose (pre-relu) -> relu fused into the psum evacuation
            for hb in range(KT):
                tp = tpsum.tile([128, 128], F32, tag="tp", name=f"tp_r{t}")
                nc.tensor.transpose(
                    tp[:], z[:, mc, hb * 128:(hb + 1) * 128], ident[:])
                rt = work.tile([128, 128], F32, tag="rt", name=f"rt_r{t}",
                               bufs=2)
                bs = slice(mc * 128, (mc + 1) * 128)
                nc.scalar.activation(reluT[0][:, hb, bs], tp[:], AF.Relu)
                nc.vector.scalar_tensor_tensor(
                    rt[:], tp[:], 0.0, reluT[0][:, hb, bs],
                    ALU.max, ALU.subtract)
                nc.gpsimd.tensor_copy(reluT[1][:, hb, bs], rt[:])

        # ---- fc2 -> y [128, MC, 64], output DMA ----
        y = work.tile([128, MC, 64], F32, tag="y", name=f"y{t}", bufs=1)
        for mc in range(2):
            yp = tpsum.tile([128, 128], F32, tag="tp", name=f"yp{t}")
            for k in range(KT):
                mm3(yp[:, 0:64],
                    [reluT[i][:, k, mc * 128:(mc + 1) * 128] for i in range(2)],
                    [wfc2[i][:, k, :] for i in range(2)],
                    start=(k == 0), stop=(k == KT - 1), n0=0, n1=64)
            nc.vector.tensor_add(y[:, mc, :], yp[:, 0:64], fc2b[:, 0:64])
        nc.sync.dma_start(
            io["ys"].ap()[t].rearrange("(mc p) d -> p mc d", p=128), y[:])

        # ---- yT pair for next step's gates0 input part ----
        if t + 1 < t_steps:
            yTn = [state.tile([65, B2], F16, tag=f"yT{i}", name=f"yTn{i}_{t}")
                   for i in range(2)]
            yTf = work.tile([64, B2], F32, tag="yTf", name=f"yTf{t}", bufs=1)
            for mc in range(2):
                ytp = tpsum.tile([128, 128], F32, tag="tp", name=f"ytp{t}")
                nc.tensor.transpose(ytp[0:64, :], y[:, mc, :], ident[:])
                nc.vector.tensor_copy(yTf[:, mc * 128:(mc + 1) * 128],
                                      ytp[0:64, 0:128])
            nc.scalar.activation(yTn[0][0:64, :], yTf[:], AF.Copy)
            rty = work.tile([64, B2], F32, tag="rty", name=f"rty{t}", bufs=1)
            nc.vector.tensor_sub(rty[:], yTf[:], yTn[0][0:64, :])
            nc.gpsimd.tensor_copy(yTn[1][0:64, :], rty[:])
            nc.vector.memset(yTn[0][64:65, :], 1.0)
            nc.vector.memset(yTn[1][64:65, :], 0.0)
            yT = yTn
            gates0_ih(g0, yT)


def build(t_steps=T_FULL, trivial_ln=True):
    key = (t_steps, trivial_ln)
    if key in _cache:
        return _cache[key]
    nc = bacc.Bacc("TRN2", target_bir_lowering=False, debug=False,
                   num_devices=NC)
    io = {}
    inputs = [
        ("wemb_hi", (65, G), F16), ("wemb_lo", (65, G), F16),
        ("whh0_hi", (H, G), F16), ("whh0_lo", (H, G), F16),
        ("wih1_hi", (H, G), F16), ("wih1_lo", (H, G), F16),
        ("whh1_hi", (H, G), F16), ("whh1_lo", (H, G), F16),
        ("wfc1_hi", (HS, H), F16), ("wfc1_lo", (HS, H), F16),
        ("wfc2_hi", (H, 64), F16), ("wfc2_lo", (H, 64), F16),
        ("b1g", (128, G), F32),
        ("fc1b", (128, H), F32), ("fc2b", (128, 64), F32),
        ("ident", (128, 128), F32),
        ("x0T_hi", (65, B2), F16), ("x0T_lo", (65, B2), F16),
        ("h0T0_hi", (H, B2), F16), ("h0T0_lo", (H, B2), F16),
        ("h1T0_hi", (H, B2), F16), ("h1T0_lo", (H, B2), F16),
        ("c00", (B2, HS), F32), ("c10", (B2, HS), F32),
    ]
    if not trivial_ln:
        inputs += [("lng", (128, H), F32), ("lnb", (128, H), F32)]
    for name, shape, dt in inputs:
        io[name] = nc.dram_tensor(name, shape, dt, kind="ExternalInput")
    io["ys"] = nc.dram_tensor("ys", (t_steps, B2, D), F32,
                              kind="ExternalOutput")
    with tile.TileContext(nc) as tc:
        with ExitStack() as ctx:
            _emit(ctx, tc, io, t_steps, trivial_ln)
    nc.compile()
    _cache[key] = (nc, io)
    return nc, io


def _split16(x):
    hi = x.astype(np.float16)
    lo = (x - hi.astype(np.float64)).astype(np.float16)
    return np.ascontiguousarray(hi), np.ascontiguousarray(lo)


def _trivial_ln(inputs):
    return bool(np.all(np.asarray(inputs["ln_g"]) == 1.0)
                and np.all(np.asarray(inputs["ln_b"]) == 0.0))


def make_in_maps(inputs, trivial_ln=True):
    f64 = lambda k: np.asarray(inputs[k]).astype(np.float64)
    emb_W, emb_b = f64("emb_W"), f64("emb_b")
    W_ih0 = f64("W_ih0")
    Wemb = emb_W @ W_ih0                     # [64, 4096]
    b0 = f64("b_ih0") + f64("b_hh0") + emb_b @ W_ih0
    b1 = f64("b_ih1") + f64("b_hh1")
    W_hh0, W_ih1, W_hh1 = f64("W_hh0"), f64("W_ih1"), f64("W_hh1")
    fc1_W, fc2_W = f64("fc1_W"), f64("fc2_W")
    x0 = f64("x_0")
    hn, cn = f64("h_n"), f64("c_n")
    ones = np.ones((1, B2), np.float64)

    bc = lambda v: np.tile(v.astype(np.float32)[None, :], (128, 1))
    in_maps = []
    for c in range(NC):
        q, m = c // 4, c % 4
        bs = slice(q * B2, (q + 1) * B2)
        cols = np.concatenate(
            [np.arange(g * H + m * HS, g * H + (m + 1) * HS) for g in range(4)])
        m_ = {}
        wemb_aug = np.vstack([Wemb[:, cols], b0[cols][None, :]])  # [65, G]
        for name, w in [("wemb", wemb_aug), ("whh0", W_hh0[:, cols]),
                        ("wih1", W_ih1[:, cols]), ("whh1", W_hh1[:, cols]),
                        ("wfc1", fc1_W[m * HS:(m + 1) * HS, :]),
                        ("wfc2", fc2_W)]:
            m_[f"{name}_hi"], m_[f"{name}_lo"] = _split16(w)
        m_["b1g"] = bc(b1[cols])
        m_["fc1b"] = bc(f64("fc1_b"))
        if not trivial_ln:
            m_["lng"] = bc(f64("ln_g"))
            m_["lnb"] = bc(f64("ln_b"))
        m_["fc2b"] = bc(f64("fc2_b"))
        m_["ident"] = np.eye(128, dtype=np.float32)
        m_["x0T_hi"], m_["x0T_lo"] = _split16(
            np.vstack([x0[bs].T, ones]))
        m_["h0T0_hi"], m_["h0T0_lo"] = _split16(hn[0][bs].T)
        m_["h1T0_hi"], m_["h1T0_lo"] = _split16(hn[1][bs].T)
        m_["c00"] = np.ascontiguousarray(
            cn[0][bs, m * HS:(m + 1) * HS].astype(np.float32))
        m_["c10"] = np.ascontiguousarray(
            cn[1][bs, m * HS:(m + 1) * HS].astype(np.float32))
        in_maps.append(m_)
    return in_maps


def kernel(**inputs):
    t_steps = int(inputs.get("forecast_window", T_FULL))
    triv = _trivial_ln(inputs)
    nc, io = build(t_steps, triv)
    in_maps = make_in_maps(inputs, triv)
    r = bass_utils.run_bass_kernel_spmd(nc, in_maps, core_ids=list(range(NC)))
    out = np.empty((B, t_steps, D), np.float32)
    for q in range(2):
        ys = r.results[q * 4]["ys"]            # [t, B2, D]
        out[q * B2:(q + 1) * B2] = ys.transpose(1, 0, 2)
    return out
